# revision 17
# baseline (speedup 1.0000x reference)
"""Trainium2 Bass kernel for nn_BaseAttention (sliding-window attention).

Full-input contract: kernel(x, Wqkv) -> [B, T, C] float32.

Sharding (8 cores): data-parallel over B (2) x tensor-parallel over head
groups (16 heads -> 4 groups of 4). Core c handles batch c//4, head group
c%4. Each core computes its QKV projection slice (768 of 3072 output rows)
and banded attention for its 4 heads; outputs are disjoint channel slices
of the final [B, T, C] tensor, so no collectives are needed.

Device-side layout (per core):
  xT  [1024, 2048]  x[b] transposed (contraction dim on partitions)
  wT  [1024, 768]   W rows (q|k|v for this head group) transposed; q part
                    pre-scaled by D**-0.5 so scores come out scaled
  msk [128, 4, 256] multiplicative 0/1 window masks, one per key chunk
                    position relative to the query-block pair
  out [2048, 256]   attention output, channels h*64+d for local heads h

Pipeline per core: QKV projection in fp32r (q/k as [head_dim, t], v as
[t, head_dim]); attention processes QUERY-BLOCK PAIRS (256 queries)
against their 4 (3 at the edges) 128-key chunks in TRANSPOSED
orientation -- scores come out as [key, query] so the exp'd tile IS the
P^T operand that P@V needs (no PE transposes of P), and the N=256 free
dim keeps fp32r matmuls at full rate. exp needs no max subtraction
(scores are bounded N(0,1) sums; softmax is shift-invariant). The
sliding window is a 0/1 multiply after exp; zeroed halves make the
full-width P^T @ [v | 1] accumulation correct for both query blocks at
once and produce the output AND the softmax denominator (65th row). One
small PE transpose flips each [65,128] half to [128,65]; a per-row
reciprocal multiply normalizes during the PSUM eviction.
"""

import os
import sys

import numpy as np

if "/opt/trn_rl_repo" not in sys.path:
    sys.path.insert(0, "/opt/trn_rl_repo")

B, T, C = 2, 2048, 1024
HEADS = 16
D = C // HEADS  # 64
WINDOW = 128
N_CORES = 8
HPC = HEADS // 4  # heads per core (4)
OPC = 3 * HPC * D  # projection output rows per core (768)

# Attention P/V dtype: "bf16" (fast) or "fp32" (precise-ish via fp32r).
PDT_NAME = os.environ.get("SA_PDT", "bf16")

_PROGRAM_CACHE = {}


def _build_program(pdt_name):
    import concourse.mybir as mybir
    from concourse import bacc
    import concourse.tile as tile
    from concourse.masks import make_identity
    from contextlib import ExitStack

    f32 = mybir.dt.float32
    f32r = mybir.dt.float32r
    bf16 = mybir.dt.bfloat16
    PDT = bf16 if pdt_name == "bf16" else f32r
    Exp = mybir.ActivationFunctionType.Exp

    nc = bacc.Bacc()
    xT_d = nc.declare_dram_parameter("xT", [C, T], f32r, isOutput=False)
    wT_d = nc.declare_dram_parameter("wT", [C, OPC], f32r, isOutput=False)
    msk_d = nc.declare_dram_parameter("msk", [128, 4, 256], f32, isOutput=False)
    out_d = nc.declare_dram_parameter("out", [T, HPC * D], f32, isOutput=True)

    CC = C // 128  # 8 contraction chunks
    TS = 512  # projection t-slice
    NS = T // TS  # 4 slices
    NB = T // 128  # 16 query blocks

    with ExitStack() as ctx:
        tc = ctx.enter_context(tile.TileContext(nc))
        const = ctx.enter_context(tc.tile_pool(name="const", bufs=1))
        xpool = ctx.enter_context(tc.tile_pool(name="xp", bufs=4))
        ppool = ctx.enter_context(tc.tile_pool(name="pp", bufs=3))
        otpool = ctx.enter_context(tc.tile_pool(name="ot", bufs=3))
        lpool = ctx.enter_context(tc.tile_pool(name="lp", bufs=8))
        qk_ps = ctx.enter_context(tc.tile_pool(name="qkps", bufs=1, space="PSUM"))
        sc_ps = ctx.enter_context(tc.tile_pool(name="scps", bufs=2, space="PSUM"))
        ov_ps = ctx.enter_context(tc.tile_pool(name="ovps", bufs=2, space="PSUM"))
        of_ps = ctx.enter_context(tc.tile_pool(name="ofps", bufs=1, space="PSUM"))

        w_sb = const.tile([128, CC, OPC], f32r)
        wT_r = wT_d.rearrange("(cc p) o -> p cc o", p=128)
        for c in range(CC):
            nc.gpsimd.dma_start(out=w_sb[:, c, :], in_=wT_r[:, c, :])
        msk_sb = const.tile([128, 4, 256], PDT)
        nc.gpsimd.dma_start(out=msk_sb, in_=msk_d[:, :, :])
        id_sb = const.tile([128, 128], f32)
        make_identity(nc, id_sb)

        q_sb = const.tile([128, 2, T], f32r)
        k_sb = const.tile([128, 2, T], f32r)
        # v packed per (key block, head) with a trailing ones column: P^T @
        # [v | 1] yields the output block and the softmax denominator at once.
        v_sb = const.tile([128, NB, HPC, D + 1], PDT)
        o_sb = const.tile([128, NB, HPC * D], f32)
        nc.vector.memset(v_sb[:, :, :, D:D + 1], 1.0)

        xT_r = xT_d.rearrange("(cc p) t -> p cc t", p=128)

        # ---- QKV projection, fp32r ----
        for s in range(NS):
            xs = xpool.tile([128, CC, TS], f32r, tag="xs")
            for c in range(CC):
                nc.sync.dma_start(
                    out=xs[:, c, :], in_=xT_r[:, c, s * TS:(s + 1) * TS]
                )
            # qT / kT: [o_part, t]; m-tiles: q0 q1 k0 k1
            for m in range(4):
                ps = qk_ps.tile([128, TS], f32, tag="ps")
                for c in range(CC):
                    nc.tensor.matmul(
                        ps,
                        lhsT=w_sb[:, c, m * 128:(m + 1) * 128],
                        rhs=xs[:, c, :],
                        start=(c == 0),
                        stop=(c == CC - 1),
                    )
                dst = (q_sb if m < 2 else k_sb)[:, m % 2, s * TS:(s + 1) * TS]
                nc.scalar.copy(dst, ps)
            # v: [t_part, o]
            for t4 in range(TS // 128):
                pv = qk_ps.tile([128, D * HPC], f32, tag="ps")
                for c in range(CC):
                    nc.tensor.matmul(
                        pv,
                        lhsT=xs[:, c, t4 * 128:(t4 + 1) * 128],
                        rhs=w_sb[:, c, 2 * D * HPC:3 * D * HPC],
                        start=(c == 0),
                        stop=(c == CC - 1),
                    )
                tb = s * (TS // 128) + t4
                nc.vector.tensor_copy(
                    v_sb[:, tb, :, 0:D], pv.rearrange("p (h d) -> p h d", h=HPC)
                )

        # ---- banded attention, transposed scores, query-block PAIRS ----
        # Each iteration handles queries [256*ip, 256*ip+256) = blocks
        # (i0, i0+1) against key chunks (i0-1 .. i0+2). Masked halves of
        # each exp'd chunk are zeroed, which makes the full-width
        # P^T @ [v|1] accumulation correct for both query blocks at once.
        for ip in range(NB // 2):
            i0 = 2 * ip
            jbs = [jb for jb in range(i0 - 1, i0 + 3) if 0 <= jb < NB]
            nch = len(jbs)
            for h in range(HPC):
                mt, po = divmod(h, 2)
                po *= 64
                # scores^T chunks: [key j (part), query pair (256 free)]
                sct = sc_ps.tile([128, 4, 256], f32, tag="sc")
                for cc2, jb in enumerate(jbs):
                    nc.tensor.matmul(
                        sct[:, cc2, :],
                        lhsT=k_sb[po:po + 64, mt, jb * 128:(jb + 1) * 128],
                        rhs=q_sb[po:po + 64, mt, i0 * 128:(i0 + 2) * 128],
                        start=True,
                        stop=True,
                    )
                p_t = ppool.tile([128, 4, 256], PDT, tag="p")
                for cc2, jb in enumerate(jbs):
                    nc.scalar.activation(p_t[:, cc2, :], sct[:, cc2, :], Exp)
                    nc.vector.tensor_mul(
                        p_t[:, cc2, :], p_t[:, cc2, :], msk_sb[:, jb - i0 + 1, :]
                    )
                # P^T @ [v | 1] -> [out^T ; l] as [65, 256] for both blocks
                ov = ov_ps.tile([65, 256], f32, tag="ov")
                for cc2, jb in enumerate(jbs):
                    nc.tensor.matmul(
                        ov,
                        lhsT=v_sb[:, jb, h, :],
                        rhs=p_t[:, cc2, :],
                        start=(cc2 == 0),
                        stop=(cc2 == nch - 1),
                    )
                ot = otpool.tile([65, 256], f32, tag="ot")
                nc.scalar.copy(ot, ov)
                for half in range(2):
                    i = i0 + half
                    of = of_ps.tile([128, 65], f32, tag="of")
                    nc.tensor.transpose(
                        of, ot[:, half * 128:(half + 1) * 128], id_sb[0:65, 0:65]
                    )
                    r_t = lpool.tile([128, 1], f32, tag="r")
                    nc.vector.reciprocal(r_t, of[:, D:D + 1])
                    nc.vector.tensor_scalar_mul(
                        o_sb[:, i, h * D:(h + 1) * D], of[:, 0:D], r_t
                    )
            nc.sync.dma_start(
                out=out_d[i0 * 128:(i0 + 2) * 128, :].rearrange(
                    "(i p) c -> p i c", p=128
                ),
                in_=o_sb[:, i0:i0 + 2, :],
            )

    nc.compile()
    return nc


def _host_inputs(x, Wqkv):
    """Per-core input maps: shard batch x head-group, pre-transpose."""
    scale = float(D) ** -0.5
    r = np.arange(128, dtype=np.float32)[:, None]
    ci = np.arange(128, dtype=np.float32)[None, :]
    # per key chunk jb relative to the query-block pair (i0, i0+1):
    # prev = allowed iff query col <= key row; next = iff col >= row
    prev = (ci <= r).astype(np.float32)
    nxt = (ci >= r).astype(np.float32)
    one = np.ones_like(prev)
    zero = np.zeros_like(prev)
    msk = np.stack(
        [
            np.concatenate([prev, zero], 1),  # jb = i0-1
            np.concatenate([one, prev], 1),   # jb = i0
            np.concatenate([nxt, one], 1),    # jb = i0+1
            np.concatenate([zero, nxt], 1),   # jb = i0+2
        ],
        axis=1,
    ).astype(np.float32)  # [128, 4, 256]

    x = np.asarray(x, dtype=np.float32)
    Wqkv = np.asarray(Wqkv, dtype=np.float32)
    xT = [np.ascontiguousarray(x[b].T) for b in range(B)]
    in_maps = []
    for core in range(N_CORES):
        b, hg = divmod(core, N_CORES // B)
        rows = slice(hg * HPC * D, (hg + 1) * HPC * D)
        wcat = np.concatenate(
            [
                Wqkv[0 * C:1 * C][rows] * scale,
                Wqkv[1 * C:2 * C][rows],
                Wqkv[2 * C:3 * C][rows],
            ],
            axis=0,
        )
        in_maps.append(
            {
                "xT": xT[b],
                "wT": np.ascontiguousarray(wcat.T),
                "msk": msk,
            }
        )
    return in_maps


def _gather(results):
    out = np.empty((B, T, C), dtype=np.float32)
    for core in range(N_CORES):
        b, hg = divmod(core, N_CORES // B)
        out[b, :, hg * HPC * D:(hg + 1) * HPC * D] = results[core]["out"]
    return out


def kernel(x, Wqkv):
    from concourse.bass_utils import run_bass_kernel_spmd

    key = PDT_NAME
    if key not in _PROGRAM_CACHE:
        _PROGRAM_CACHE[key] = _build_program(key)
    nc = _PROGRAM_CACHE[key]
    in_maps = _host_inputs(x, Wqkv)
    res = run_bass_kernel_spmd(nc, in_maps, list(range(N_CORES)))
    return _gather(res.results)
